# revision 3
# baseline (speedup 1.0000x reference)
"""FBGCN layer on 8 Trainium2 NeuronCores — v3.

Math (reference):
    Lhp = (d_inv @ lap) @ d_inv
    Hh  = Lhp @ relu(x @ W_high)
    Hl  = GCNConv(x, edge_index, W_conv, b_conv)
    out = aL * Hl + aH * Hh

v3 vs v2:
  * ONE AllGather per gather (2 total instead of 4): each collective call
    costs a serialized ~5-20us floor, and a single [P, 4*D] staging layout
    makes the cc_out readback land in 2KB-per-partition contiguous runs
    (~2.5x the DMA efficiency of the old 512B fragments).
  * Natural global chunk order everywhere (no slot permutation needed:
    single-AG output is rank-major = global row order).
  * Stage B is chunk-major (k outer, m inner): consumes x/R/d progressively,
    so it starts ~5us earlier, overlapped with the d load.
  * xw (= x @ W_conv) is deferred until after B: it covers AG1's flight.
    Stage C runs plain-fp8 (no DoubleRow): same aT bytes (2MB), double the
    PE coverage for the AG gaps. C0 covers AG1 readback; C1-3 cover AG2.
  * Readbacks split in two rank-halves so D/E start on the first half.
  * Output stored bf16 (host upcasts): |out| ~ aH*|Hh| >> quantization.
  * DMA ring split (as v2): bulk loads on sync ring in need-order;
    latency-critical staging/readback/out stores on the scalar ring.
"""

import numpy as np
import ml_dtypes

import concourse.bass as bass
import concourse.mybir as mybir
import concourse.tile as tile
from concourse import bacc
from concourse.bass_utils import run_bass_kernel_spmd

N = 4096
D = 256
E = 131072
NCORES = 8
RPC = N // NCORES          # rows per core = 512
KC = N // 128              # contraction chunks = 32
MT = RPC // 128            # output row tiles per core = 4
P = 128

BF16 = mybir.dt.bfloat16
F32 = mybir.dt.float32
FP8 = mybir.dt.float8e4
nbf16 = ml_dtypes.bfloat16
nfp8 = ml_dtypes.float8_e4m3

RELU = mybir.ActivationFunctionType.Relu

# readback halves: slots (= global chunks) 0..15 are ranks 0-3, 16..31 ranks 4-7
HALF1 = list(range(KC // 2))
HALF2 = list(range(KC // 2, KC))


def build_program(repeat: int = 1, ablate: frozenset = frozenset(), serial: bool = True):
    """Build the SPMD per-core program (identical on all cores)."""
    nc = bacc.Bacc(num_devices=NCORES)

    # ---- I/O ----  (matrix inputs come host-pre-transposed to [P, kc*m])
    xT = nc.declare_dram_parameter("xT", [P, 2 * N], BF16, isOutput=False)
    Whc = nc.declare_dram_parameter("Whc", [P, 2 * 2 * D], BF16, isOutput=False)
    dT = nc.declare_dram_parameter("dT", [P, KC * RPC], BF16, isOutput=False)
    lT = nc.declare_dram_parameter("lT", [P, KC * RPC], BF16, isOutput=False)
    aT = nc.declare_dram_parameter("aT", [P, KC * RPC], FP8, isOutput=False)
    bL = nc.declare_dram_parameter("bL", [P, D], F32, isOutput=False)
    out = nc.declare_dram_parameter("out", [RPC, D], BF16, isOutput=True)

    # collective bounce buffers: one gather = one collective of [P, MT*D]
    cc_in = {}
    cc_out = {}
    for g in (1, 2):
        cc_in[g] = nc.dram_tensor(f"cc{g}_in", [P, MT * D], BF16)
        cc_out[g] = nc.dram_tensor(
            f"cc{g}_out", [NCORES * P, MT * D], BF16, addr_space="Shared"
        )

    dT_v = dT.rearrange("p (kc m) -> p kc m", kc=KC)
    lT_v = lT.rearrange("p (kc m) -> p kc m", kc=KC)
    aT_v = aT.rearrange("p (kc m) -> p kc m", kc=KC)
    xT_v = xT.rearrange("p (kc m) -> p kc m", kc=2)
    Whc_v = Whc.rearrange("p (kc m) -> p kc m", kc=2)
    cc_in_v = {k: v.rearrange("p (mt m) -> p mt m", mt=MT) for k, v in cc_in.items()}
    # readback: rank r partition p holds chunks 4r..4r+3 as 4 contiguous D-cols
    cc_out_v = {
        k: v.rearrange("(rc p) (mt m) -> p rc mt m", p=P, mt=MT)
        for k, v in cc_out.items()
    }
    out_v = out.rearrange("(mt p) m -> p mt m", p=P)

    NCHUNK = 4
    kk = KC // NCHUNK
    replica_groups = [list(range(NCORES))]

    def allgather(g):
        nc.gpsimd.collective_compute(
            "AllGather",
            mybir.AluOpType.bypass,
            replica_groups=replica_groups,
            ins=[cc_in[g][:]],
            outs=[cc_out[g][:]],
        )

    with tile.TileContext(nc) as tc:
        with (
            tc.tile_pool(name="const", bufs=1) as cpool,
            tc.tile_pool(name="bigmat", bufs=1) as bigpool,
            tc.tile_pool(name="acts", bufs=1) as apool,
            tc.tile_pool(name="psum", bufs=8, space="PSUM") as pspool,
            tc.tile_pool(name="outp", bufs=2) as opool,
        ):
            for _rep in range(repeat):
                if serial and _rep > 0:
                    # full flush between iterations: slope == single-shot latency
                    tc.strict_bb_all_engine_barrier()

                # ---- bulk loads, sync ring, in need-order: W, x, d, a, l ----
                xT_sb = cpool.tile([P, 2, N], BF16, tag="xT")
                Whc_sb = cpool.tile([P, 2, 2 * D], BF16, tag="Whc")
                bL_sb = cpool.tile([P, D], F32, tag="bL")
                d_sb = bigpool.tile([P, KC, RPC], BF16, tag="d")
                a_sb = bigpool.tile([P, KC, RPC], FP8, tag="a")
                l_sb = bigpool.tile([P, KC, RPC], BF16, tag="l")
                nc.sync.dma_start(out=Whc_sb[:], in_=Whc_v)
                nc.sync.dma_start(out=bL_sb[:], in_=bL[:])
                # x in quarters so stage A starts after ~0.5MB lands
                for mh in range(2):
                    s = slice(mh * (N // 2), (mh + 1) * (N // 2))
                    for k in range(2):
                        nc.sync.dma_start(out=xT_sb[:, k, s], in_=xT_v[:, k, s])
                if "load" not in ablate:
                    for c in range(NCHUNK):
                        s = slice(c * kk, (c + 1) * kk)
                        nc.sync.dma_start(out=d_sb[:, s, :], in_=dT_v[:, s, :])
                    for c in range(NCHUNK):
                        s = slice(c * kk, (c + 1) * kk)
                        nc.sync.dma_start(out=a_sb[:, s, :], in_=aT_v[:, s, :])
                    for c in range(NCHUNK):
                        s = slice(c * kk, (c + 1) * kk)
                        nc.sync.dma_start(out=l_sb[:, s, :], in_=lT_v[:, s, :])
                else:
                    nc.sync.dma_start(out=d_sb[:, :1, :64], in_=dT_v[:, :1, :64])
                    nc.sync.dma_start(out=a_sb[:, :1, :128], in_=aT_v[:, :1, :128])
                    nc.sync.dma_start(out=l_sb[:, :1, :64], in_=lT_v[:, :1, :64])

                # ---- stage A (R half): R = relu(x @ aH*W_high), bf16 ----
                R_sb = apool.tile([P, KC, D], BF16, tag="R")
                xw_sb = apool.tile([P, KC, D], FP8, tag="xw")
                if "A" in ablate:
                    nc.sync.dma_start(out=R_sb[:, :1, :64], in_=dT_v[:, :1, :64])
                    nc.sync.dma_start(out=xw_sb[:, :1, :128], in_=aT_v[:, :1, :128])
                if "A" not in ablate:
                    for m in range(KC):
                        psA = pspool.tile([P, D], F32, tag="ps", name=f"psA{m}_{_rep}")
                        for k in range(2):
                            nc.tensor.matmul(
                                out=psA[:],
                                lhsT=xT_sb[:, k, m * P:(m + 1) * P],
                                rhs=Whc_sb[:, k, :D],
                                start=(k == 0),
                                stop=(k == 1),
                            )
                        nc.scalar.activation(R_sb[:, m, :], psA[:], RELU)

                def gather_store(g, pst, m):
                    t = opool.tile([P, D], BF16, tag="gst", name=f"gs{g}{m}_{_rep}")
                    nc.vector.tensor_copy(t[:], pst[:])
                    nc.scalar.dma_start(out=cc_in_v[g][:, m, :], in_=t[:])

                def gather_load(g, half, dst_sb):
                    # rank half -> 16 chunk slots; 2KB contiguous per partition
                    sl = slice(0, KC // 2) if half == 1 else slice(KC // 2, KC)
                    rc = slice(0, NCORES // 2) if half == 1 else slice(NCORES // 2, NCORES)
                    nc.scalar.dma_start(
                        out=dst_sb[:, sl, :].rearrange("p (rc mt) m -> p rc mt m", mt=MT),
                        in_=cc_out_v[g][:, rc, :, :],
                    )

                Hl_sb = opool.tile([P, MT, D], F32, tag="Hl")

                # ---- stage B (chunk-major): P1_loc = d_inv[rows] @ R ----
                if "B" not in ablate:
                    psB = {}
                    for m in range(MT):
                        psB[m] = pspool.tile([P, D], F32, tag="ps", name=f"psB{m}_{_rep}")
                    for c in range(KC):
                        for m in range(MT):
                            nc.tensor.matmul(
                                out=psB[m][:],
                                lhsT=d_sb[:, c, m * P:(m + 1) * P],
                                rhs=R_sb[:, c, :],
                                start=(c == 0),
                                stop=(c == KC - 1),
                            )
                    for m in range(MT):
                        gather_store(1, psB[m], m)
                    if "AG1" not in ablate:
                        allgather(1)

                # ---- stage A (xw half, deferred): xw = fp8(x @ W_conv) ----
                # covers AG1 flight; C0 covers the readback
                if "A" not in ablate:
                    for m in range(KC):
                        psX = pspool.tile([P, D], F32, tag="ps", name=f"psX{m}_{_rep}")
                        for k in range(2):
                            nc.tensor.matmul(
                                out=psX[:],
                                lhsT=xT_sb[:, k, m * P:(m + 1) * P],
                                rhs=Whc_sb[:, k, D:],
                                start=(k == 0),
                                stop=(k == 1),
                            )
                        nc.vector.tensor_copy(xw_sb[:, m, :], psX[:])

                def stage_c_mtile(m):
                    # plain fp8 matmuls (same rate as bf16, half the SBUF bytes)
                    ps = pspool.tile([P, D], F32, tag="ps", name=f"psC{m}_{_rep}")
                    for c in range(KC):
                        nc.tensor.matmul(
                            out=ps[:],
                            lhsT=a_sb[:, c, m * P:(m + 1) * P],
                            rhs=xw_sb[:, c, :],
                            start=(c == 0),
                            stop=(c == KC - 1),
                        )
                    nc.vector.tensor_add(Hl_sb[:, m, :], ps[:], bL_sb[:])

                if "C" not in ablate:
                    stage_c_mtile(0)
                else:
                    for m in range(MT):
                        nc.vector.tensor_copy(Hl_sb[:, m, :], bL_sb[:])

                # ---- stage D: P2_loc = lap[rows] @ P1, split on rb halves ----
                P1_sb = apool.tile([P, KC, D], BF16, tag="P1")
                if "AG1" not in ablate or True:
                    gather_load(1, 1, P1_sb)
                    gather_load(1, 2, P1_sb)
                psD = {}
                if "D" not in ablate:
                    for m in range(MT):
                        psD[m] = pspool.tile([P, D], F32, tag="ps", name=f"psD{m}_{_rep}")
                    for half in (HALF1, HALF2):
                        for m in range(MT):
                            for i, c in enumerate(half):
                                nc.tensor.matmul(
                                    out=psD[m][:],
                                    lhsT=l_sb[:, c, m * P:(m + 1) * P],
                                    rhs=P1_sb[:, c, :],
                                    start=(half is HALF1 and i == 0),
                                    stop=(half is HALF2 and i == len(half) - 1),
                                )
                    for m in range(MT):
                        gather_store(2, psD[m], m)
                    if "AG2" not in ablate:
                        allgather(2)

                # ---- stage C (rest): covers AG2 flight + readback ----
                if "C" not in ablate:
                    stage_c_mtile(1)
                    stage_c_mtile(2)
                    stage_c_mtile(3)

                # ---- stage E: out = Hl + d_inv[rows] @ P2, split on rb halves ----
                P2_sb = apool.tile([P, KC, D], BF16, tag="P2")
                gather_load(2, 1, P2_sb)
                gather_load(2, 2, P2_sb)
                if "E" not in ablate:
                    psE = {}
                    for m in range(MT):
                        psE[m] = pspool.tile([P, D], F32, tag="ps", name=f"psE{m}_{_rep}")
                    for half in (HALF1, HALF2):
                        for m in range(MT):
                            for i, c in enumerate(half):
                                nc.tensor.matmul(
                                    out=psE[m][:],
                                    lhsT=d_sb[:, c, m * P:(m + 1) * P],
                                    rhs=P2_sb[:, c, :],
                                    start=(half is HALF1 and i == 0),
                                    stop=(half is HALF2 and i == len(half) - 1),
                                )
                    for m in range(MT):
                        o_sb = opool.tile([P, D], BF16, tag="osb", name=f"os{m}_{_rep}")
                        nc.vector.tensor_add(o_sb[:], psE[m][:], Hl_sb[:, m, :])
                        nc.scalar.dma_start(out=out_v[:, m, :], in_=o_sb[:])

    nc.finalize()
    return nc


def prep_inputs(x, edge_index, lap, d_inv, W_high, W_conv, b_conv, aL, aH):
    """Host-side sharding/layout: build per-core input maps."""
    x = np.asarray(x, dtype=np.float32)
    lap = np.asarray(lap, dtype=np.float32)
    d_inv = np.asarray(d_inv, dtype=np.float32)
    W_high = np.asarray(W_high, dtype=np.float32)
    W_conv = np.asarray(W_conv, dtype=np.float32)
    b_conv = np.asarray(b_conv, dtype=np.float32)
    aLs = float(np.asarray(aL).reshape(-1)[0])
    aHs = float(np.asarray(aH).reshape(-1)[0])
    src = np.asarray(edge_index[0], dtype=np.int64)
    dst = np.asarray(edge_index[1], dtype=np.int64)

    # symmetric GCN normalization (with self-loops) folded into a dense adjacency
    deg = np.bincount(dst, minlength=N).astype(np.float32) + 1.0
    dis = 1.0 / np.sqrt(deg)
    A_T = np.zeros((N, N), dtype=np.float32)           # A_T[src, dst]
    np.add.at(A_T, (src, dst), aLs * dis[src] * dis[dst])
    A_T[np.arange(N), np.arange(N)] += aLs * dis * dis

    def to_pkm(arrT):
        # [K, M] -> [P, kc*M]: element (p, c*M + m) = arrT[128*c + p, m]
        Kdim, Mdim = arrT.shape
        kc = Kdim // P
        a = arrT.reshape(kc, P, Mdim)
        return np.ascontiguousarray(a.transpose(1, 0, 2).reshape(P, kc * Mdim))

    xT = to_pkm(np.ascontiguousarray(x.T).astype(nbf16))
    Whc = to_pkm(np.concatenate([W_high * aHs, W_conv], axis=1).astype(nbf16))
    bLb = np.broadcast_to(aLs * b_conv, (P, D)).astype(np.float32).copy()
    dT_full = np.ascontiguousarray(d_inv.T).astype(nbf16)
    lT_full = np.ascontiguousarray(lap.T).astype(nbf16)
    aT_full = np.clip(A_T, -240, 240).astype(nfp8)

    in_maps = []
    for i in range(NCORES):
        sl = slice(i * RPC, (i + 1) * RPC)
        in_maps.append({
            "xT": xT,
            "Whc": Whc,
            "dT": to_pkm(dT_full[:, sl]),
            "lT": to_pkm(lT_full[:, sl]),
            "aT": to_pkm(aT_full[:, sl]),
            "bL": bLb,
        })
    return in_maps


def kernel(x, edge_index, lap, d_inv, W_high, W_conv, b_conv, aL, aH):
    in_maps = prep_inputs(x, edge_index, lap, d_inv, W_high, W_conv, b_conv, aL, aH)
    nc = build_program()
    res = run_bass_kernel_spmd(nc, in_maps, list(range(NCORES)))
    return np.concatenate(
        [res.results[i]["out"] for i in range(NCORES)], axis=0
    ).astype(np.float32)
